# revision 1
# baseline (speedup 1.0000x reference)
"""Trainium2 Bass kernel for retrieval-KNN attention (nn_MAM_68418829025563).

Math (reference):
    query  = x @ w1.T + b1                       # [B, D]
    key    = keys @ w2.T + b2                    # [B, K, D]
    scores = (query . key) / sqrt(D)             # [B, K]
    attn   = softmax(scores, axis=-1)
    out    = 0.5*x + 0.5 * sum_k attn[:,k] * values[:,k,:]

Algebraic refactor (avoids the B*K*D*D key projection; factor K less compute):
    scores[b,k] = (q2[b] . keys[b,k] + s0[b]) / sqrt(D)
    q2 = x @ W + c   with W = w1.T @ w2, c = b1 @ w2      (host-folded weights)
    s0 = x @ u + c0  with u = w1.T @ b2, c0 = b1 . b2

Device mapping (pure data parallel, batch sharded over 8 cores):
  - PE: x transpose, q2/s0 projection, exp-scores shear to block-diagonal,
        attention-weighted value combine (block-diagonal matmul trick).
  - DVE: per-sample products q2*keys (ACT accumulates the dot via the
         activation accumulator), reciprocal, final merge.
  - ACT: PSUM->SBUF copies, exp (+fused denominator accumulation).
  - Kernel is memory-bound: streams keys+values (128 MB/core) at HBM rate.
"""

import math
import os

import numpy as np

B, K, D = 8192, 32, 512
N_CORES = 8
BS = B // N_CORES        # samples per core
P = 128                  # partition tile (samples per b-tile)
NBT = BS // P            # b-tiles per core
NCH = D // P             # contraction chunks of 128
KC = 16                  # keys per DMA chunk
GC = 8                   # value groups (4 samples each) per DMA chunk
NG = P // 4              # groups of 4 samples per b-tile = 32
INV_SQRT_D = 1.0 / math.sqrt(D)
ALPHA = 0.5

_NC_CACHE = {}
LAST_RESULTS = None


def _build_nc():
    import concourse.bass as bass
    import concourse.tile as tile
    from concourse import bacc, mybir

    f32 = mybir.dt.float32
    nc = bacc.Bacc(
        "TRN2",
        target_bir_lowering=False,
        debug=False,
        enable_asserts=False,
        num_devices=N_CORES,
    )

    xs = nc.dram_tensor("xs", [BS, D], f32, kind="ExternalInput").ap()
    keys = nc.dram_tensor("keys", [BS, K, D], f32, kind="ExternalInput").ap()
    values = nc.dram_tensor("values", [BS, K, D], f32, kind="ExternalInput").ap()
    wt = nc.dram_tensor("wt", [P, NCH, D], f32, kind="ExternalInput").ap()
    cvec = nc.dram_tensor("cvec", [1, D], f32, kind="ExternalInput").ap()
    ut = nc.dram_tensor("ut", [P, NCH], f32, kind="ExternalInput").ap()
    c0s = nc.dram_tensor("c0s", [1, 1], f32, kind="ExternalInput").ap()
    smat = nc.dram_tensor("smat", [K, 4, P], f32, kind="ExternalInput").ap()
    ident = nc.dram_tensor("ident", [P, P], f32, kind="ExternalInput").ap()
    out_d = nc.dram_tensor("out", [BS, D], f32, kind="ExternalOutput").ap()

    # values as [(b k), d] rows, partition-major view: vflat2[p, n, d] is flat
    # row n*128+p, so tile n holds 4 consecutive samples' (k, d) rows.
    vflat2 = values.flatten_outer_dims().rearrange("(n p) d -> p n d", p=P)

    mult = mybir.AluOpType.mult
    add = mybir.AluOpType.add

    with tile.TileContext(nc) as tc:
        with (
            tc.tile_pool(name="singles", bufs=1) as singles,
            tc.tile_pool(name="kpool", bufs=2) as kpool,
            tc.tile_pool(name="vpool", bufs=2) as vpool,
            tc.tile_pool(name="xpool", bufs=2) as xpool,
            tc.tile_pool(name="spool", bufs=2) as spool,
            tc.tile_pool(name="opool", bufs=2) as opool,
            tc.tile_pool(name="ps_xt", bufs=1, space="PSUM") as ps_xt,
            tc.tile_pool(name="ps_q2", bufs=1, space="PSUM") as ps_q2,
            tc.tile_pool(name="ps_s0", bufs=1, space="PSUM") as ps_s0,
            tc.tile_pool(name="ps_et", bufs=1, space="PSUM") as ps_et,
            tc.tile_pool(name="ps_l2", bufs=1, space="PSUM") as ps_l2,
            tc.tile_pool(name="ps_cb", bufs=2, space="PSUM") as ps_cb,
        ):
            # --- constants (loaded once) ---
            wt_sb = singles.tile([P, NCH, D], f32)
            nc.sync.dma_start(out=wt_sb, in_=wt)
            cvec_sb = singles.tile([1, D], f32)
            nc.sync.dma_start(out=cvec_sb, in_=cvec)
            ut_sb = singles.tile([P, NCH], f32)
            nc.sync.dma_start(out=ut_sb, in_=ut)
            c0_sb = singles.tile([1, 1], f32)
            nc.sync.dma_start(out=c0_sb, in_=c0s)
            smat_sb = singles.tile([K, 4, P], f32)
            nc.sync.dma_start(out=smat_sb, in_=smat)
            ident_sb = singles.tile([P, P], f32)
            nc.sync.dma_start(out=ident_sb, in_=ident)
            ones_sb = singles.tile([1, P], f32)
            nc.vector.memset(ones_sb, 1.0)
            # G2: per-group zero-padded [128, 64] lhsT tiles for the combine.
            # Group g = 16*beta + j lives at cols [1024*beta + 64*j, +64);
            # its only nonzero columns are 4*j + s (s = 0..3), so the single
            # memset here stays valid across all b-tiles.
            g2_sb = singles.tile([P, 2 * NG * P // 4], f32)  # [128, 2048]
            nc.vector.memset(g2_sb, 0.0)
            g2_view = g2_sb.rearrange("p (b r) -> p b r", b=2)

            for t in range(NBT):
                b0 = t * P

                # --- load x tile, halve it for the final merge ---
                x_tile = xpool.tile([P, D], f32)
                nc.sync.dma_start(out=x_tile, in_=xs[b0 : b0 + P, :])
                x_half = xpool.tile([P, D], f32)
                nc.scalar.mul(out=x_half, in_=x_tile, mul=ALPHA)

                # --- xT via PE transpose ---
                xt_ps = ps_xt.tile([P, NCH, P], f32)
                for j in range(NCH):
                    nc.tensor.transpose(
                        xt_ps[:, j, :], x_tile[:, j * P : (j + 1) * P], ident_sb
                    )
                xt_sb = spool.tile([P, NCH, P], f32, tag="xt_sb")
                nc.scalar.copy(out=xt_sb, in_=xt_ps)

                # --- q2 = x @ W + c ; s0 = x @ u + c0 (PSUM accumulation) ---
                q2_ps = ps_q2.tile([P, D], f32)
                s0_ps = ps_s0.tile([P, 1], f32)
                for j in range(NCH):
                    nc.tensor.matmul(
                        q2_ps, xt_sb[:, j, :], wt_sb[:, j, :],
                        start=(j == 0), stop=False,
                    )
                for j in range(NCH):
                    nc.tensor.matmul(
                        s0_ps, xt_sb[:, j, :], ut_sb[:, j : j + 1],
                        start=(j == 0), stop=False,
                    )
                nc.tensor.matmul(q2_ps, ones_sb, cvec_sb, start=False, stop=True)
                nc.tensor.matmul(s0_ps, ones_sb, c0_sb, start=False, stop=True)
                # q2 pre-scaled by 1/sqrt(D) during the PSUM->SBUF move.
                q2_sb = spool.tile([P, D], f32, tag="q2_sb")
                nc.scalar.mul(out=q2_sb, in_=q2_ps, mul=INV_SQRT_D)
                s0_sb = spool.tile([P, 1], f32, tag="s0_sb")
                nc.scalar.mul(out=s0_sb, in_=s0_ps, mul=INV_SQRT_D)

                # --- scores[b,k] = (q2 . keys[b,k]) / sqrt(D) ---
                # (tensor_tensor_reduce is rejected by this runtime, so:
                #  DVE does the elementwise product, ACT reduces it via the
                #  activation accumulator — the two pipeline across k.)
                scores_sb = spool.tile([P, K], f32, tag="scores")
                for kc0 in range(0, K, KC):
                    ktile = kpool.tile([P, KC, D], f32)
                    nc.sync.dma_start(
                        out=ktile, in_=keys[b0 : b0 + P, kc0 : kc0 + KC, :]
                    )
                    for kl in range(KC):
                        k = kc0 + kl
                        scratch = spool.tile([P, D], f32, tag="scratch")
                        nc.vector.tensor_mul(scratch, q2_sb, ktile[:, kl, :])
                        nc.scalar.activation(
                            out=scratch, in_=scratch,
                            func=mybir.ActivationFunctionType.Copy,
                            accum_out=scores_sb[:, k : k + 1],
                        )

                # --- softmax pieces: E = exp(scores + s0s), denom = sum_k E ---
                e_sb = spool.tile([P, K], f32, tag="e_sb")
                denom_sb = spool.tile([P, 1], f32, tag="denom")
                nc.scalar.activation(
                    out=e_sb, in_=scores_sb,
                    func=mybir.ActivationFunctionType.Exp,
                    bias=s0_sb,
                    accum_out=denom_sb,
                )
                denom2_sb = spool.tile([P, 1], f32, tag="denom2")
                nc.scalar.mul(out=denom2_sb, in_=denom_sb, mul=1.0 / ALPHA)
                rhalf_sb = spool.tile([P, 1], f32, tag="rhalf")
                nc.vector.reciprocal(out=rhalf_sb, in_=denom2_sb)

                # --- shear E into block-diagonal L2 via PE ---
                # L2[32s+k, 32s+g] = E[4g+s, k]; all other entries written 0.
                et_ps = ps_et.tile([K, P], f32)
                nc.tensor.transpose(et_ps, e_sb, ident_sb)
                et_sb = spool.tile([K, P], f32, tag="et_sb")
                nc.scalar.copy(out=et_sb, in_=et_ps)
                et_view = et_sb.rearrange("k (g s4) -> k s4 g", s4=4)
                l2_ps = ps_l2.tile([P, P], f32)
                for s in range(4):
                    nc.tensor.matmul(
                        l2_ps[:, 32 * s : 32 * (s + 1)],
                        smat_sb[:, s, :],
                        et_view[:, s, :],
                        start=True, stop=True,
                    )
                # Scatter L2's nonzero columns into the pre-zeroed G2 tiles:
                # G2 col 1024*beta + 68*j + s  <-  L2 col 32*s + 16*beta + j.
                l2_view = l2_ps.rearrange("p (s4 b q) -> p q b s4", s4=4, b=2)
                for j in range(NG // 2):
                    nc.scalar.copy(
                        out=g2_view[:, :, 68 * j : 68 * j + 4],
                        in_=l2_view[:, j, :, :],
                    )

                # --- combine = sum_k E * values via block-diag matmuls ---
                # Half-block beta accumulates its 16 groups into rows
                # [64*beta, 64*beta+64) of comb_ps.
                comb_ps = ps_cb.tile([P, D], f32)
                for vc in range(0, NG, GC):
                    vtile = vpool.tile([P, GC, D], f32)
                    nc.sync.dma_start(
                        out=vtile,
                        in_=vflat2[:, NG * t + vc : NG * t + vc + GC, :],
                    )
                    for gi in range(GC):
                        g = vc + gi
                        beta, j = divmod(g, NG // 2)
                        nc.tensor.matmul(
                            comb_ps[64 * beta : 64 * (beta + 1), :],
                            g2_sb[:, 1024 * beta + 64 * j : 1024 * beta + 64 * (j + 1)],
                            vtile[:, gi, :],
                            start=(j == 0), stop=(j == NG // 2 - 1),
                        )

                # --- out = 0.5*x + (0.5/denom)*comb ---
                out_sb = opool.tile([P, D], f32)
                nc.vector.scalar_tensor_tensor(
                    out=out_sb,
                    in0=comb_ps,
                    scalar=rhalf_sb,
                    in1=x_half,
                    op0=mult,
                    op1=add,
                )
                nc.sync.dma_start(out=out_d[b0 : b0 + P, :], in_=out_sb)

    nc.compile()
    return nc


def _get_nc():
    if "nc" not in _NC_CACHE:
        _NC_CACHE["nc"] = _build_nc()
    return _NC_CACHE["nc"]


def _host_consts(w1, b1, w2, b2):
    w1 = np.asarray(w1, np.float32)
    b1 = np.asarray(b1, np.float32)
    w2 = np.asarray(w2, np.float32)
    b2 = np.asarray(b2, np.float32)
    W = w1.T @ w2                       # [D, D]
    c = b1 @ w2                         # [D]
    u = w1.T @ b2                       # [D]
    c0 = float(b1 @ b2)
    wt = np.ascontiguousarray(W.reshape(NCH, P, D).transpose(1, 0, 2))
    cvec = np.ascontiguousarray(c.reshape(1, D))
    ut = np.ascontiguousarray(u.reshape(NCH, P).T)
    c0s = np.full((1, 1), c0, np.float32)
    smat = np.zeros((K, 4, P), np.float32)
    for k in range(K):
        for s in range(4):
            smat[k, s, 32 * s + k] = 1.0
    identm = np.eye(P, dtype=np.float32)
    return wt, cvec, ut, c0s, smat, identm


def kernel(x, keys, values, w1, b1, w2, b2):
    global LAST_RESULTS
    from concourse import bass_utils

    x = np.ascontiguousarray(np.asarray(x, np.float32))
    keys = np.ascontiguousarray(np.asarray(keys, np.float32))
    values = np.ascontiguousarray(np.asarray(values, np.float32))
    wt, cvec, ut, c0s, smat, identm = _host_consts(w1, b1, w2, b2)

    nc = _get_nc()
    in_maps = []
    for ci in range(N_CORES):
        sl = slice(ci * BS, (ci + 1) * BS)
        in_maps.append(
            dict(
                xs=x[sl],
                keys=keys[sl],
                values=values[sl],
                wt=wt,
                cvec=cvec,
                ut=ut,
                c0s=c0s,
                smat=smat,
                ident=identm,
            )
        )
    res = bass_utils.run_bass_kernel_spmd(
        nc, in_maps, core_ids=list(range(N_CORES))
    )
    LAST_RESULTS = res
    return np.concatenate([r["out"] for r in res.results], axis=0)



# revision 27
# speedup vs baseline: 2.0354x; 2.0354x over previous
"""Trainium2 Bass kernel for retrieval-KNN attention (nn_MAM_68418829025563).

Math (reference):
    query  = x @ w1.T + b1                       # [B, D]
    key    = keys @ w2.T + b2                    # [B, K, D]
    scores = (query . key) / sqrt(D)             # [B, K]
    attn   = softmax(scores, axis=-1)
    out    = 0.5*x + 0.5 * sum_k attn[:,k] * values[:,k,:]

Algebraic refactors:
  - key projection folded into the query side: scores[b,k] ∝ q2[b].keys[b,k]
    with q2 = x @ W + c, W = w1.T @ w2, c = b1 @ w2 (host-folded). The
    s0[b] = x.u + c0 term is constant across k, and softmax is
    shift-invariant, so it is dropped entirely.
  - The kernel is memory-bound (streams keys+values); inputs are converted
    to fp16 on the host, halving HBM traffic. rel-err stays ~1e-3, well
    inside the 2e-2 budget.

Device mapping (pure data parallel, batch sharded over 8 cores):
  - SP queue: all streaming loads (keys prefetched one b-tile ahead, then
    values); Pool/SWDGE: output stores (their long waits must not block
    the load queue). PE: transposes, q2 projection, shear, and the
    block-diagonal attention-weighted value combine (fp16, 1 cycle/row).
  - DVE: fp16 products (2x mode) + 12/32 of the score reductions; ACT:
    20/32 reductions via the activation accumulator, exp, PSUM->SBUF moves.
  - Output merge+store is split per 64-row half-block so the last store
    chain after the final value chunk is short; the last tile's value
    chunks taper (8,8,8,7,1).
"""

import math

import numpy as np

B, K, D = 8192, 32, 512
N_CORES = 8
BS = B // N_CORES        # samples per core
P = 128                  # partition tile (samples per b-tile)
NBT = BS // P            # b-tiles per core
NCH = D // P             # contraction chunks of 128
KC = 8                   # keys per DMA chunk
NKC = K // KC            # key chunks per tile
GC = 8                   # value groups (4 samples each) per DMA chunk
NG = P // 4              # groups of 4 samples per b-tile = 32
INV_SQRT_D = 1.0 / math.sqrt(D)
ALPHA = 0.5

_NC_CACHE = {}
LAST_RESULTS = None


def _build_nc():
    import concourse.bass as bass
    import concourse.tile as tile
    from concourse import bacc, mybir
    from concourse.ap import AP

    f32 = mybir.dt.float32
    f16 = mybir.dt.float16
    nc = bacc.Bacc(
        "TRN2",
        target_bir_lowering=False,
        debug=False,
        enable_asserts=False,
        num_devices=N_CORES,
    )

    xs = nc.dram_tensor("xs", [BS, D], f16, kind="ExternalInput").ap()
    keys = nc.dram_tensor("keys", [BS, K, D], f16, kind="ExternalInput").ap()
    values = nc.dram_tensor("values", [BS, K, D], f16, kind="ExternalInput").ap()
    # wt [P, NCH*D] | ident [P, P] packed in one fp16 tensor (fewer startup DMAs)
    wtpack = nc.dram_tensor(
        "wtpack", [P, NCH * D + P], f16, kind="ExternalInput"
    ).ap()
    cvec = nc.dram_tensor("cvec", [1, D], f16, kind="ExternalInput").ap()
    smat = nc.dram_tensor("smat", [K, 4, P], f16, kind="ExternalInput").ap()
    out_d = nc.dram_tensor("out", [BS, D], f32, kind="ExternalOutput").ap()

    # values as [(b k), d] rows, partition-major view: vflat2[p, n, d] is flat
    # row n*128+p, so tile n holds 4 consecutive samples' (k, d) rows.
    vflat2 = values.flatten_outer_dims().rearrange("(n p) d -> p n d", p=P)

    mult = mybir.AluOpType.mult
    add = mybir.AluOpType.add

    with tile.TileContext(nc) as tc:
        with (
            tc.tile_pool(name="singles", bufs=1) as singles,
            tc.tile_pool(name="kpool", bufs=3 * NKC) as kpool,
            tc.tile_pool(name="vpool", bufs=8) as vpool,
            tc.tile_pool(name="xpool", bufs=3) as xpool,
            tc.tile_pool(name="spool", bufs=2) as spool,
            tc.tile_pool(name="scrpool", bufs=8) as scrpool,
            tc.tile_pool(name="opool", bufs=2) as opool,
            tc.tile_pool(name="ps_xt", bufs=1, space="PSUM") as ps_xt,
            tc.tile_pool(name="ps_q2", bufs=1, space="PSUM") as ps_q2,
            tc.tile_pool(name="ps_et", bufs=1, space="PSUM") as ps_et,
            tc.tile_pool(name="ps_l2", bufs=1, space="PSUM") as ps_l2,
            tc.tile_pool(name="ps_cb", bufs=2, space="PSUM") as ps_cb,
        ):
            # --- constants (loaded once) ---
            wtpack_sb = singles.tile([P, NCH * D + P], f16)
            nc.scalar.dma_start(out=wtpack_sb, in_=wtpack)
            wt_sb = wtpack_sb[:, 0 : NCH * D].rearrange("p (c d) -> p c d", c=NCH)
            ident_sb = wtpack_sb[:, NCH * D : NCH * D + P]
            cvec_sb = singles.tile([1, D], f16)
            nc.scalar.dma_start(out=cvec_sb, in_=cvec)
            smat_sb = singles.tile([K, 4, P], f16)
            nc.sync.dma_start(out=smat_sb, in_=smat)
            ones_sb = singles.tile([1, P], f16)
            nc.vector.memset(ones_sb, 1.0)
            # G2: per-group zero-padded [128, 64] lhsT tiles for the combine.
            # Group g = 16*beta + j lives at cols [1024*beta + 64*j, +64);
            # its only nonzero columns are 4*j + s (s = 0..3), so the single
            # memset here stays valid across all b-tiles.
            g2_sb = singles.tile([P, 2 * NG * P // 4], f16)  # [128, 2048]
            nc.vector.memset(g2_sb, 0.0)

            # Streaming loads, all on the SP queue. Keys (and x) for tile t+1
            # are issued BEFORE tile t's values, so softmax(t) is finished
            # by the time tile t's values arrive and the combine can consume
            # each value chunk immediately.
            x_tiles = [None] * NBT
            k_tiles = [[None] * NKC for _ in range(NBT)]
            x_halves = [None] * NBT
            q2s = [None] * NBT
            merge_args = [None] * NBT

            def load_tile_kx(t):
                b0 = t * P
                x_tiles[t] = xpool.tile([P, D], f16, tag="x", name="x_tile")
                nc.sync.dma_start(out=x_tiles[t], in_=xs[b0 : b0 + P, :])
                for c in range(NKC):
                    kt = kpool.tile([P, KC, D], f16, name="ktile")
                    nc.sync.dma_start(
                        out=kt, in_=keys[b0 : b0 + P, c * KC : (c + 1) * KC, :]
                    )
                    k_tiles[t][c] = kt

            def q2_section(t):
                # xT via PE transpose; q2 = (x @ W + c)/sqrt(D) as fp16.
                # Emitted one tile ahead of tile t-1's shear+combine so the
                # in-order PE queue never gates the next tile's scores.
                x_tile = x_tiles[t]
                x_half = xpool.tile([P, D], f32, tag="xh", name="x_half")
                nc.scalar.mul(out=x_half, in_=x_tile, mul=ALPHA)
                x_halves[t] = x_half
                xt_ps = ps_xt.tile([P, NCH, P], f16, name="xt_ps")
                for j in range(NCH):
                    nc.tensor.transpose(
                        xt_ps[:, j, :], x_tile[:, j * P : (j + 1) * P], ident_sb
                    )
                xt_sb = spool.tile([P, NCH, P], f16, tag="xt_sb", name="xt_sb")
                nc.scalar.copy(out=xt_sb, in_=xt_ps)
                q2_ps = ps_q2.tile([P, D], f32, name="q2_ps")
                for j in range(NCH):
                    nc.tensor.matmul(
                        q2_ps, xt_sb[:, j, :], wt_sb[:, j, :],
                        start=(j == 0), stop=False,
                    )
                nc.tensor.matmul(q2_ps, ones_sb, cvec_sb, start=False, stop=True)
                q2_sb = spool.tile([P, D], f16, tag="q2_sb", name="q2_sb")
                nc.scalar.mul(out=q2_sb, in_=q2_ps, mul=INV_SQRT_D)
                q2s[t] = q2_sb

            def emit_merges(t):
                # out rows = 0.5*x + (1/denom)*comb, one stt+store per
                # half-block. Deferred to after tile t+1's score muls so the
                # stt's wait on combine(t) never head-of-line-blocks the DVE
                # queue. Stores go via Pool/SWDGE so their wait on out_sb
                # can't block the SP load queue; the very last store rides
                # the (by then idle) SP queue.
                comb_ps, out_sb, rhalf_sb, xh, b0 = merge_args[t]
                for beta in (0, 1):
                    r0, r1 = 64 * beta, 64 * (beta + 1)
                    nc.vector.scalar_tensor_tensor(
                        out=out_sb[r0:r1, :],
                        in0=comb_ps[r0:r1, :],
                        scalar=rhalf_sb[r0:r1, :],
                        in1=xh[r0:r1, :],
                        op0=mult,
                        op1=add,
                    )
                    eng = nc.sync if (t == NBT - 1 and beta == 1) else nc.gpsimd
                    eng.dma_start(
                        out=out_d[b0 + r0 : b0 + r1, :],
                        in_=out_sb[r0:r1, :],
                    )

            scores_sbs = [None] * NBT

            def scores_section(t):
                # scores[b,k] = q2 . keys[b,k], chasing this tile's key-chunk
                # arrivals (they stream one window ahead of its values). DVE
                # does the fp16 product (2x mode); the length-512 reduce is
                # split 20/32 on ACT (activation accumulator) and 12/32 on
                # DVE (tensor_reduce) to balance engine load.
                q2_sb = q2s[t]
                scores_sb = spool.tile([P, K], f32, tag="scores", name="scores")
                scores_sbs[t] = scores_sb
                for c in range(NKC):
                    ktile = k_tiles[t][c]
                    for kl in range(KC):
                        k = c * KC + kl
                        scratch = scrpool.tile([P, D], f16, tag="scratch",
                                               name="scratch")
                        nc.vector.tensor_mul(scratch, q2_sb, ktile[:, kl, :])
                        if k % 8 < 5:
                            nc.scalar.activation(
                                out=scratch, in_=scratch,
                                func=mybir.ActivationFunctionType.Copy,
                                accum_out=scores_sb[:, k : k + 1],
                            )
                        else:
                            nc.vector.tensor_reduce(
                                out=scores_sb[:, k : k + 1],
                                in_=scratch,
                                axis=mybir.AxisListType.X,
                                op=add,
                            )

            load_tile_kx(0)
            q2_section(0)
            scores_section(0)

            for t in range(NBT):
                b0 = t * P
                x_half = x_halves[t]
                scores_sb = scores_sbs[t]

                # next tile's loads + q2 (PE/ACT run it during exp(t)'s wait)
                if t + 1 < NBT:
                    load_tile_kx(t + 1)
                    q2_section(t + 1)

                # --- softmax pieces: E = exp(scores), denom = sum_k E ---
                # Emitted before scores(t+1) so ACT fires exp(t) immediately
                # (tile t's scores finished last window).
                e_sb = spool.tile([P, K], f16, tag="e_sb")
                denom_sb = spool.tile([P, 1], f32, tag="denom")
                nc.scalar.activation(
                    out=e_sb, in_=scores_sb,
                    func=mybir.ActivationFunctionType.Exp,
                    accum_out=denom_sb,
                )
                rhalf_sb = spool.tile([P, 1], f32, tag="rhalf")
                nc.vector.reciprocal(out=rhalf_sb, in_=denom_sb)

                # --- shear E into block-diagonal G2 via PE ---
                # L2[32s+k, 32s+c] = E[4c+s, k] (zeros elsewhere); scattered
                # into G2 col 1024*beta + 68*j + s <- L2 col 32*s + 16*beta
                # + j with ONE strided-AP copy per half (the pattern is
                # affine in (j, s)). The 0.5 output weight folds into the
                # scatter (comb = 0.5*sum E*v, rhalf = 1/denom).
                et_ps = ps_et.tile([K, P], f16)
                nc.tensor.transpose(et_ps, e_sb, ident_sb)
                et_sb = spool.tile([K, P], f16, tag="et_sb")
                nc.scalar.copy(out=et_sb, in_=et_ps)
                et_view = et_sb.rearrange("k (g s4) -> k s4 g", s4=4)
                l2_ps = ps_l2.tile([P, P], f32)
                for s in range(4):
                    nc.tensor.matmul(
                        l2_ps[:, 32 * s : 32 * (s + 1)],
                        smat_sb[:, s, :],
                        et_view[:, s, :],
                        start=True, stop=True,
                    )
                g2_pstr = g2_sb.ap[0][0]
                l2_pstr = l2_ps.ap[0][0]
                for beta in (0, 1):
                    nc.scalar.mul(
                        out=AP(
                            g2_sb.tensor,
                            g2_sb.offset + 1024 * beta,
                            [[g2_pstr, P], [68, NG // 2], [1, 4]],
                        ),
                        in_=AP(
                            l2_ps.tensor,
                            l2_ps.offset + 16 * beta,
                            [[l2_pstr, P], [1, NG // 2], [32, 4]],
                        ),
                        mul=ALPHA,
                    )

                # next tile's scores after this tile's exp/shear (ACT order),
                # then the previous tile's deferred merges+stores (DVE order:
                # after scores(t+1) muls so their wait on combine can't
                # head-of-line-block the queue).
                if t + 1 < NBT:
                    scores_section(t + 1)
                if t > 0:
                    emit_merges(t - 1)

                # --- combine = sum_k E * values via block-diag matmuls ---
                # Half-block beta accumulates its 16 groups into rows
                # [64*beta, 64*beta+64) of comb_ps; each half merges+stores as
                # soon as it is final. The last tile tapers its final chunks.
                comb_ps = ps_cb.tile([P, D], f32)
                out_sb = opool.tile([P, D], f32)
                if t == NBT - 1:
                    vchunks = [(0, 8), (8, 16), (16, 24), (24, 31), (31, 32)]
                else:
                    vchunks = [(0, 8), (8, 16), (16, 24), (24, 32)]
                for vc0, vc1 in vchunks:
                    vtile = vpool.tile([P, vc1 - vc0, D], f16)
                    nc.sync.dma_start(
                        out=vtile,
                        in_=vflat2[:, NG * t + vc0 : NG * t + vc1, :],
                    )
                    for gi in range(vc1 - vc0):
                        g = vc0 + gi
                        beta, j = divmod(g, NG // 2)
                        nc.tensor.matmul(
                            comb_ps[64 * beta : 64 * (beta + 1), :],
                            g2_sb[
                                :, 1024 * beta + 64 * j : 1024 * beta + 64 * (j + 1)
                            ],
                            vtile[:, gi, :],
                            start=(j == 0), stop=(j == NG // 2 - 1),
                        )
                merge_args[t] = (comb_ps, out_sb, rhalf_sb, x_half, b0)
                if t == NBT - 1:
                    emit_merges(t)

    nc.compile()
    return nc


def _get_nc():
    if "nc" not in _NC_CACHE:
        _NC_CACHE["nc"] = _build_nc()
    return _NC_CACHE["nc"]


def _host_consts(w1, b1, w2, b2):
    w1 = np.asarray(w1, np.float32)
    b1 = np.asarray(b1, np.float32)
    w2 = np.asarray(w2, np.float32)
    b2 = np.asarray(b2, np.float32)
    W = w1.T @ w2                       # [D, D]
    c = b1 @ w2                         # [D]
    wt = W.reshape(NCH, P, D).transpose(1, 0, 2).reshape(P, NCH * D)
    identm = np.eye(P, dtype=np.float32)
    wtpack = np.ascontiguousarray(
        np.concatenate([wt, identm], axis=1), np.float16
    )
    cvec = np.ascontiguousarray(c.reshape(1, D), np.float16)
    smat = np.zeros((K, 4, P), np.float16)
    for k in range(K):
        for s in range(4):
            smat[k, s, 32 * s + k] = 1.0
    return wtpack, cvec, smat


def kernel(x, keys, values, w1, b1, w2, b2):
    global LAST_RESULTS
    from concourse import bass_utils

    x = np.ascontiguousarray(np.asarray(x, np.float16))
    keys = np.ascontiguousarray(np.asarray(keys, np.float16))
    values = np.ascontiguousarray(np.asarray(values, np.float16))
    wtpack, cvec, smat = _host_consts(w1, b1, w2, b2)

    nc = _get_nc()
    in_maps = []
    for ci in range(N_CORES):
        sl = slice(ci * BS, (ci + 1) * BS)
        in_maps.append(
            dict(
                xs=x[sl],
                keys=keys[sl],
                values=values[sl],
                wtpack=wtpack,
                cvec=cvec,
                smat=smat,
            )
        )
    res = bass_utils.run_bass_kernel_spmd(
        nc, in_maps, core_ids=list(range(N_CORES))
    )
    LAST_RESULTS = res
    return np.concatenate([r["out"] for r in res.results], axis=0)


# revision 33
# speedup vs baseline: 2.0470x; 1.0057x over previous
"""Trainium2 Bass kernel for retrieval-KNN attention (nn_MAM_68418829025563).

Math (reference):
    query  = x @ w1.T + b1                       # [B, D]
    key    = keys @ w2.T + b2                    # [B, K, D]
    scores = (query . key) / sqrt(D)             # [B, K]
    attn   = softmax(scores, axis=-1)
    out    = 0.5*x + 0.5 * sum_k attn[:,k] * values[:,k,:]

Algebraic refactors:
  - key projection folded into the query side: scores[b,k] ∝ q2[b].keys[b,k]
    with q2 = x @ W + c, W = w1.T @ w2, c = b1 @ w2 (host-folded). The
    s0[b] = x.u + c0 term is constant across k, and softmax is
    shift-invariant, so it is dropped entirely.
  - The kernel is memory-bound (streams keys+values); inputs are converted
    to fp16 on the host, halving HBM traffic. rel-err stays ~1e-3, well
    inside the 2e-2 budget.

Device mapping (pure data parallel, batch sharded over 8 cores):
  - SP queue: all streaming loads (keys prefetched one b-tile ahead, then
    values); Pool/SWDGE: output stores (their long waits must not block
    the load queue). PE: transposes, q2 projection, shear, and the
    block-diagonal attention-weighted value combine (fp16, 1 cycle/row).
  - DVE: fp16 products (2x mode) + 12/32 of the score reductions; ACT:
    20/32 reductions via the activation accumulator, exp, PSUM->SBUF moves.
  - Output merge+store is split per 64-row half-block so the last store
    chain after the final value chunk is short; the last tile's value
    chunks taper (8,8,8,7,1).
"""

import math

import numpy as np

B, K, D = 8192, 32, 512
N_CORES = 8
BS = B // N_CORES        # samples per core
P = 128                  # partition tile (samples per b-tile)
NBT = BS // P            # b-tiles per core
NCH = D // P             # contraction chunks of 128
KC = 8                   # keys per DMA chunk
NKC = K // KC            # key chunks per tile
GC = 8                   # value groups (4 samples each) per DMA chunk
NG = P // 4              # groups of 4 samples per b-tile = 32
INV_SQRT_D = 1.0 / math.sqrt(D)
ALPHA = 0.5

_NC_CACHE = {}
LAST_RESULTS = None


def _build_nc():
    import concourse.bass as bass
    import concourse.tile as tile
    from concourse import bacc, mybir
    from concourse.ap import AP

    f32 = mybir.dt.float32
    f16 = mybir.dt.float16
    nc = bacc.Bacc(
        "TRN2",
        target_bir_lowering=False,
        debug=False,
        enable_asserts=False,
        num_devices=N_CORES,
    )

    xs = nc.dram_tensor("xs", [BS, D], f16, kind="ExternalInput").ap()
    keys = nc.dram_tensor("keys", [BS, K, D], f16, kind="ExternalInput").ap()
    values = nc.dram_tensor("values", [BS, K, D], f16, kind="ExternalInput").ap()
    # wt [P, NCH*D] | ident [P, P] packed in one fp16 tensor (fewer startup DMAs)
    wtpack = nc.dram_tensor(
        "wtpack", [P, NCH * D + P], f16, kind="ExternalInput"
    ).ap()
    cvec = nc.dram_tensor("cvec", [1, D], f16, kind="ExternalInput").ap()
    smat = nc.dram_tensor("smat", [K, 4, P], f16, kind="ExternalInput").ap()
    out_d = nc.dram_tensor("out", [BS, D], f32, kind="ExternalOutput").ap()

    # values as [(b k), d] rows, partition-major view: vflat2[p, n, d] is flat
    # row n*128+p, so tile n holds 4 consecutive samples' (k, d) rows.
    vflat2 = values.flatten_outer_dims().rearrange("(n p) d -> p n d", p=P)

    mult = mybir.AluOpType.mult
    add = mybir.AluOpType.add

    with tile.TileContext(nc) as tc:
        with (
            tc.tile_pool(name="singles", bufs=1) as singles,
            tc.tile_pool(name="kpool", bufs=3 * NKC) as kpool,
            tc.tile_pool(name="vpool", bufs=8) as vpool,
            tc.tile_pool(name="xpool", bufs=3) as xpool,
            tc.tile_pool(name="spool", bufs=2) as spool,
            tc.tile_pool(name="scrpool", bufs=8) as scrpool,
            tc.tile_pool(name="opool", bufs=2) as opool,
            tc.tile_pool(name="ps_xt", bufs=1, space="PSUM") as ps_xt,
            tc.tile_pool(name="ps_q2", bufs=1, space="PSUM") as ps_q2,
            tc.tile_pool(name="ps_et", bufs=1, space="PSUM") as ps_et,
            tc.tile_pool(name="ps_l2", bufs=1, space="PSUM") as ps_l2,
            tc.tile_pool(name="ps_cb", bufs=3, space="PSUM") as ps_cb,
        ):
            # --- constants (loaded once) ---
            wtpack_sb = singles.tile([P, NCH * D + P], f16)
            nc.scalar.dma_start(out=wtpack_sb, in_=wtpack)
            wt_sb = wtpack_sb[:, 0 : NCH * D].rearrange("p (c d) -> p c d", c=NCH)
            ident_sb = wtpack_sb[:, NCH * D : NCH * D + P]
            cvec_sb = singles.tile([1, D], f16)
            nc.scalar.dma_start(out=cvec_sb, in_=cvec)
            smat_sb = singles.tile([K, 4, P], f16)
            nc.sync.dma_start(out=smat_sb, in_=smat)
            ones_sb = singles.tile([1, P], f16)
            nc.vector.memset(ones_sb, 1.0)
            # G2: per-group zero-padded [128, 64] lhsT tiles for the combine.
            # Group g = 16*beta + j lives at cols [1024*beta + 64*j, +64);
            # its only nonzero columns are 4*j + s (s = 0..3), so the single
            # memset here stays valid across all b-tiles.
            g2_sb = singles.tile([P, 2 * NG * P // 4], f16)  # [128, 2048]
            nc.vector.memset(g2_sb, 0.0)

            # Streaming loads, all on the SP queue. Keys (and x) for tile t+1
            # are issued BEFORE tile t's values, so softmax(t) is finished
            # by the time tile t's values arrive and the combine can consume
            # each value chunk immediately.
            x_tiles = [None] * NBT
            k_tiles = [[None] * NKC for _ in range(NBT)]
            x_halves = [None] * NBT
            q2s = [None] * NBT
            merge_args = [None] * NBT
            comb_insts = [None] * NBT

            def load_tile_kx(t):
                b0 = t * P
                x_tiles[t] = xpool.tile([P, D], f16, tag="x", name="x_tile")
                nc.sync.dma_start(out=x_tiles[t], in_=xs[b0 : b0 + P, :])
                for c in range(NKC):
                    kt = kpool.tile([P, KC, D], f16, name="ktile")
                    nc.sync.dma_start(
                        out=kt, in_=keys[b0 : b0 + P, c * KC : (c + 1) * KC, :]
                    )
                    k_tiles[t][c] = kt

            def q2_section(t):
                # xT via PE transpose; q2 = (x @ W + c)/sqrt(D) as fp16.
                # Emitted one tile ahead of tile t-1's shear+combine so the
                # in-order PE queue never gates the next tile's scores.
                x_tile = x_tiles[t]
                x_half = xpool.tile([P, D], f32, tag="xh", name="x_half")
                nc.scalar.mul(out=x_half, in_=x_tile, mul=ALPHA)
                x_halves[t] = x_half
                xt_ps = ps_xt.tile([P, NCH, P], f16, name="xt_ps")
                for j in range(NCH):
                    nc.tensor.transpose(
                        xt_ps[:, j, :], x_tile[:, j * P : (j + 1) * P], ident_sb
                    )
                xt_sb = spool.tile([P, NCH, P], f16, tag="xt_sb", name="xt_sb")
                nc.scalar.copy(out=xt_sb, in_=xt_ps)
                q2_ps = ps_q2.tile([P, D], f32, name="q2_ps")
                for j in range(NCH):
                    nc.tensor.matmul(
                        q2_ps, xt_sb[:, j, :], wt_sb[:, j, :],
                        start=(j == 0), stop=False,
                    )
                nc.tensor.matmul(q2_ps, ones_sb, cvec_sb, start=False, stop=True)
                q2_sb = spool.tile([P, D], f16, tag="q2_sb", name="q2_sb")
                nc.scalar.mul(out=q2_sb, in_=q2_ps, mul=INV_SQRT_D)
                q2s[t] = q2_sb

            def emit_merges(t):
                # out rows = 0.5*x + (1/denom)*comb, one stt+store per
                # half-block. Deferred to after tile t+1's score muls so the
                # stt's wait on combine(t) never head-of-line-blocks the DVE
                # queue. Stores go via Pool/SWDGE so their wait on out_sb
                # can't block the SP load queue; the very last store rides
                # the (by then idle) SP queue.
                comb_ps, out_sb, rhalf_sb, xh, b0 = merge_args[t]
                for beta in (0, 1):
                    r0, r1 = 64 * beta, 64 * (beta + 1)
                    nc.vector.scalar_tensor_tensor(
                        out=out_sb[r0:r1, :],
                        in0=comb_ps[r0:r1, :],
                        scalar=rhalf_sb[r0:r1, :],
                        in1=xh[r0:r1, :],
                        op0=mult,
                        op1=add,
                    )
                    eng = nc.sync if (t == NBT - 1 and beta == 1) else nc.gpsimd
                    eng.dma_start(
                        out=out_d[b0 + r0 : b0 + r1, :],
                        in_=out_sb[r0:r1, :],
                    )

            scores_sbs = [None] * NBT

            def scores_section(t):
                # scores[b,k] = q2 . keys[b,k], chasing this tile's key-chunk
                # arrivals (they stream one window ahead of its values). DVE
                # does the fp16 product (2x mode); the length-512 reduce is
                # split 20/32 on ACT (activation accumulator) and 12/32 on
                # DVE (tensor_reduce) to balance engine load.
                q2_sb = q2s[t]
                scores_sb = spool.tile([P, K], f32, tag="scores", name="scores")
                scores_sbs[t] = scores_sb
                for c in range(NKC):
                    ktile = k_tiles[t][c]
                    for kl in range(KC):
                        k = c * KC + kl
                        scratch = scrpool.tile([P, D], f16, tag="scratch",
                                               name="scratch")
                        nc.vector.tensor_mul(scratch, q2_sb, ktile[:, kl, :])
                        if k % 8 < 5:
                            nc.scalar.activation(
                                out=scratch, in_=scratch,
                                func=mybir.ActivationFunctionType.Copy,
                                accum_out=scores_sb[:, k : k + 1],
                            )
                        else:
                            nc.vector.tensor_reduce(
                                out=scores_sb[:, k : k + 1],
                                in_=scratch,
                                axis=mybir.AxisListType.X,
                                op=add,
                            )

            load_tile_kx(0)
            q2_section(0)
            scores_section(0)

            for t in range(NBT):
                b0 = t * P
                x_half = x_halves[t]
                scores_sb = scores_sbs[t]

                # next tile's loads + q2 (PE/ACT run it during exp(t)'s wait)
                if t + 1 < NBT:
                    load_tile_kx(t + 1)
                    q2_section(t + 1)

                # --- softmax pieces: E = exp(scores), denom = sum_k E ---
                # Emitted before scores(t+1) so ACT fires exp(t) immediately
                # (tile t's scores finished last window).
                e_sb = spool.tile([P, K], f16, tag="e_sb")
                denom_sb = spool.tile([P, 1], f32, tag="denom")
                nc.scalar.activation(
                    out=e_sb, in_=scores_sb,
                    func=mybir.ActivationFunctionType.Exp,
                    accum_out=denom_sb,
                )
                rhalf_sb = spool.tile([P, 1], f32, tag="rhalf")
                nc.vector.reciprocal(out=rhalf_sb, in_=denom_sb)

                # --- shear E into block-diagonal G2 via PE ---
                # L2[32s+k, 32s+c] = E[4c+s, k] (zeros elsewhere); scattered
                # into G2 col 1024*beta + 68*j + s <- L2 col 32*s + 16*beta
                # + j with ONE strided-AP copy per half (the pattern is
                # affine in (j, s)). The 0.5 output weight folds into the
                # scatter (comb = 0.5*sum E*v, rhalf = 1/denom).
                et_ps = ps_et.tile([K, P], f16)
                et_tr = nc.tensor.transpose(et_ps, e_sb, ident_sb)
                if t == NBT - 1 and comb_insts[t - 1] is not None:
                    # Pin the last tile's shear behind the previous tile's
                    # combine in the in-order PE stream: the list scheduler
                    # otherwise hoists it (its v1 DMA timing runs late) and
                    # the exp(t) wait stalls PE for ~3.5us.
                    tile.add_dep_helper(
                        et_tr.ins, comb_insts[t - 1].ins, sync=True,
                        reason="last-tile shear after prev combine",
                    )
                et_sb = spool.tile([K, P], f16, tag="et_sb")
                nc.scalar.copy(out=et_sb, in_=et_ps)
                et_view = et_sb.rearrange("k (g s4) -> k s4 g", s4=4)
                l2_ps = ps_l2.tile([P, P], f32)
                for s in range(4):
                    nc.tensor.matmul(
                        l2_ps[:, 32 * s : 32 * (s + 1)],
                        smat_sb[:, s, :],
                        et_view[:, s, :],
                        start=True, stop=True,
                    )
                g2_pstr = g2_sb.ap[0][0]
                l2_pstr = l2_ps.ap[0][0]
                for beta in (0, 1):
                    nc.scalar.mul(
                        out=AP(
                            g2_sb.tensor,
                            g2_sb.offset + 1024 * beta,
                            [[g2_pstr, P], [68, NG // 2], [1, 4]],
                        ),
                        in_=AP(
                            l2_ps.tensor,
                            l2_ps.offset + 16 * beta,
                            [[l2_pstr, P], [1, NG // 2], [32, 4]],
                        ),
                        mul=ALPHA,
                    )

                # next tile's scores after this tile's exp/shear (ACT order),
                # then the previous tile's deferred merges+stores (DVE order:
                # after scores(t+1) muls so their wait on combine can't
                # head-of-line-block the queue).
                if t + 1 < NBT:
                    scores_section(t + 1)
                if t > 0:
                    emit_merges(t - 1)

                # --- combine = sum_k E * values via block-diag matmuls ---
                # Half-block beta accumulates its 16 groups into rows
                # [64*beta, 64*beta+64) of comb_ps; each half merges+stores as
                # soon as it is final. The last tile tapers its final chunks.
                comb_ps = ps_cb.tile([P, D], f32)
                out_sb = opool.tile([P, D], f32)
                if t == NBT - 1:
                    vchunks = [(0, 8), (8, 16), (16, 24), (24, 31), (31, 32)]
                else:
                    vchunks = [(0, 8), (8, 16), (16, 24), (24, 32)]
                for vc0, vc1 in vchunks:
                    vtile = vpool.tile([P, vc1 - vc0, D], f16)
                    nc.sync.dma_start(
                        out=vtile,
                        in_=vflat2[:, NG * t + vc0 : NG * t + vc1, :],
                    )
                    for gi in range(vc1 - vc0):
                        g = vc0 + gi
                        beta, j = divmod(g, NG // 2)
                        comb_insts[t] = nc.tensor.matmul(
                            comb_ps[64 * beta : 64 * (beta + 1), :],
                            g2_sb[
                                :, 1024 * beta + 64 * j : 1024 * beta + 64 * (j + 1)
                            ],
                            vtile[:, gi, :],
                            start=(j == 0), stop=(j == NG // 2 - 1),
                        )
                merge_args[t] = (comb_ps, out_sb, rhalf_sb, x_half, b0)
                if t == NBT - 1:
                    emit_merges(t)

    nc.compile()
    return nc


def _get_nc():
    if "nc" not in _NC_CACHE:
        _NC_CACHE["nc"] = _build_nc()
    return _NC_CACHE["nc"]


def _host_consts(w1, b1, w2, b2):
    w1 = np.asarray(w1, np.float32)
    b1 = np.asarray(b1, np.float32)
    w2 = np.asarray(w2, np.float32)
    b2 = np.asarray(b2, np.float32)
    W = w1.T @ w2                       # [D, D]
    c = b1 @ w2                         # [D]
    wt = W.reshape(NCH, P, D).transpose(1, 0, 2).reshape(P, NCH * D)
    identm = np.eye(P, dtype=np.float32)
    wtpack = np.ascontiguousarray(
        np.concatenate([wt, identm], axis=1), np.float16
    )
    cvec = np.ascontiguousarray(c.reshape(1, D), np.float16)
    smat = np.zeros((K, 4, P), np.float16)
    for k in range(K):
        for s in range(4):
            smat[k, s, 32 * s + k] = 1.0
    return wtpack, cvec, smat


def kernel(x, keys, values, w1, b1, w2, b2):
    global LAST_RESULTS
    from concourse import bass_utils

    x = np.ascontiguousarray(np.asarray(x, np.float16))
    keys = np.ascontiguousarray(np.asarray(keys, np.float16))
    values = np.ascontiguousarray(np.asarray(values, np.float16))
    wtpack, cvec, smat = _host_consts(w1, b1, w2, b2)

    nc = _get_nc()
    in_maps = []
    for ci in range(N_CORES):
        sl = slice(ci * BS, (ci + 1) * BS)
        in_maps.append(
            dict(
                xs=x[sl],
                keys=keys[sl],
                values=values[sl],
                wtpack=wtpack,
                cvec=cvec,
                smat=smat,
            )
        )
    res = bass_utils.run_bass_kernel_spmd(
        nc, in_maps, core_ids=list(range(N_CORES))
    )
    LAST_RESULTS = res
    return np.concatenate([r["out"] for r in res.results], axis=0)


# revision 36
# speedup vs baseline: 2.0604x; 1.0065x over previous
"""Trainium2 Bass kernel for retrieval-KNN attention (nn_MAM_68418829025563).

Math (reference):
    query  = x @ w1.T + b1                       # [B, D]
    key    = keys @ w2.T + b2                    # [B, K, D]
    scores = (query . key) / sqrt(D)             # [B, K]
    attn   = softmax(scores, axis=-1)
    out    = 0.5*x + 0.5 * sum_k attn[:,k] * values[:,k,:]

Algebraic refactors:
  - key projection folded into the query side: scores[b,k] ∝ q2[b].keys[b,k]
    with q2 = x @ W + c, W = w1.T @ w2, c = b1 @ w2 (host-folded). The
    s0[b] = x.u + c0 term is constant across k, and softmax is
    shift-invariant, so it is dropped entirely.
  - The kernel is memory-bound (streams keys+values); inputs are converted
    to fp16 on the host, halving HBM traffic. rel-err stays ~1e-3, well
    inside the 2e-2 budget.

Device mapping (pure data parallel, batch sharded over 8 cores):
  - SP queue: all streaming loads (keys prefetched one b-tile ahead, then
    values); Pool/SWDGE: output stores (their long waits must not block
    the load queue). PE: transposes, q2 projection, shear, and the
    block-diagonal attention-weighted value combine (fp16, 1 cycle/row).
  - DVE: fp16 products (2x mode) + 12/32 of the score reductions; ACT:
    20/32 reductions via the activation accumulator, exp, PSUM->SBUF moves.
  - Output merge+store is split per 64-row half-block so the last store
    chain after the final value chunk is short; the last tile's value
    chunks taper (8,8,8,7,1).
"""

import math

import numpy as np

B, K, D = 8192, 32, 512
N_CORES = 8
BS = B // N_CORES        # samples per core
P = 128                  # partition tile (samples per b-tile)
NBT = BS // P            # b-tiles per core
NCH = D // P             # contraction chunks of 128
KC = 8                   # keys per DMA chunk
NKC = K // KC            # key chunks per tile
GC = 8                   # value groups (4 samples each) per DMA chunk
NG = P // 4              # groups of 4 samples per b-tile = 32
INV_SQRT_D = 1.0 / math.sqrt(D)
ALPHA = 0.5

_NC_CACHE = {}
LAST_RESULTS = None


def _build_nc():
    import concourse.bass as bass
    import concourse.tile as tile
    from concourse import bacc, mybir
    from concourse.ap import AP

    f32 = mybir.dt.float32
    f16 = mybir.dt.float16
    nc = bacc.Bacc(
        "TRN2",
        target_bir_lowering=False,
        debug=False,
        enable_asserts=False,
        num_devices=N_CORES,
    )

    xs = nc.dram_tensor("xs", [BS, D], f16, kind="ExternalInput").ap()
    keys = nc.dram_tensor("keys", [BS, K, D], f16, kind="ExternalInput").ap()
    values = nc.dram_tensor("values", [BS, K, D], f16, kind="ExternalInput").ap()
    # wt [P, NCH*D] | ident [P, P] packed in one fp16 tensor (fewer startup DMAs)
    wtpack = nc.dram_tensor(
        "wtpack", [P, NCH * D + P], f16, kind="ExternalInput"
    ).ap()
    cvec = nc.dram_tensor("cvec", [1, D], f16, kind="ExternalInput").ap()
    smat = nc.dram_tensor("smat", [K, 4, P], f16, kind="ExternalInput").ap()
    out_d = nc.dram_tensor("out", [BS, D], f32, kind="ExternalOutput").ap()

    # values as [(b k), d] rows, partition-major view: vflat2[p, n, d] is flat
    # row n*128+p, so tile n holds 4 consecutive samples' (k, d) rows.
    vflat2 = values.flatten_outer_dims().rearrange("(n p) d -> p n d", p=P)

    mult = mybir.AluOpType.mult
    add = mybir.AluOpType.add

    with tile.TileContext(nc) as tc:
        with (
            tc.tile_pool(name="singles", bufs=1) as singles,
            tc.tile_pool(name="kpool", bufs=3 * NKC) as kpool,
            tc.tile_pool(name="vpool", bufs=8) as vpool,
            tc.tile_pool(name="xpool", bufs=3) as xpool,
            tc.tile_pool(name="spool", bufs=2) as spool,
            tc.tile_pool(name="scrpool", bufs=8) as scrpool,
            tc.tile_pool(name="opool", bufs=2) as opool,
            tc.tile_pool(name="ps_xt", bufs=1, space="PSUM") as ps_xt,
            tc.tile_pool(name="ps_q2", bufs=1, space="PSUM") as ps_q2,
            tc.tile_pool(name="ps_et", bufs=1, space="PSUM") as ps_et,
            tc.tile_pool(name="ps_l2", bufs=1, space="PSUM") as ps_l2,
            tc.tile_pool(name="ps_cb", bufs=3, space="PSUM") as ps_cb,
        ):
            # --- constants (loaded once) ---
            wtpack_sb = singles.tile([P, NCH * D + P], f16)
            nc.scalar.dma_start(out=wtpack_sb, in_=wtpack)
            wt_sb = wtpack_sb[:, 0 : NCH * D].rearrange("p (c d) -> p c d", c=NCH)
            ident_sb = wtpack_sb[:, NCH * D : NCH * D + P]
            cvec_sb = singles.tile([1, D], f16)
            nc.scalar.dma_start(out=cvec_sb, in_=cvec)
            smat_sb = singles.tile([K, 4, P], f16)
            nc.sync.dma_start(out=smat_sb, in_=smat)
            ones_sb = singles.tile([1, P], f16)
            nc.vector.memset(ones_sb, 1.0)
            # G2: per-group zero-padded [128, 64] lhsT tiles for the combine.
            # Group g = 16*beta + j lives at cols [1024*beta + 64*j, +64);
            # its only nonzero columns are 4*j + s (s = 0..3), so the single
            # memset here stays valid across all b-tiles.
            g2_sb = singles.tile([P, 2 * NG * P // 4], f16)  # [128, 2048]
            nc.vector.memset(g2_sb, 0.0)

            # Streaming loads, all on the SP queue. Keys (and x) for tile t+1
            # are issued BEFORE tile t's values, so softmax(t) is finished
            # by the time tile t's values arrive and the combine can consume
            # each value chunk immediately.
            x_tiles = [None] * NBT
            k_tiles = [[None] * NKC for _ in range(NBT)]
            x_halves = [None] * NBT
            q2s = [None] * NBT
            merge_args = [None] * NBT
            comb_insts = [None] * NBT

            def load_tile_kx(t):
                b0 = t * P
                x_tiles[t] = xpool.tile([P, D], f16, tag="x", name="x_tile")
                nc.sync.dma_start(out=x_tiles[t], in_=xs[b0 : b0 + P, :])
                for c in range(NKC):
                    kt = kpool.tile([P, KC, D], f16, name="ktile")
                    nc.sync.dma_start(
                        out=kt, in_=keys[b0 : b0 + P, c * KC : (c + 1) * KC, :]
                    )
                    k_tiles[t][c] = kt

            def q2_section(t):
                # xT via PE transpose; q2 = (x @ W + c)/sqrt(D) as fp16.
                # Emitted one tile ahead of tile t-1's shear+combine so the
                # in-order PE queue never gates the next tile's scores.
                x_tile = x_tiles[t]
                x_half = xpool.tile([P, D], f32, tag="xh", name="x_half")
                nc.scalar.mul(out=x_half, in_=x_tile, mul=ALPHA)
                x_halves[t] = x_half
                xt_ps = ps_xt.tile([P, NCH, P], f16, name="xt_ps")
                for j in range(NCH):
                    nc.tensor.transpose(
                        xt_ps[:, j, :], x_tile[:, j * P : (j + 1) * P], ident_sb
                    )
                xt_sb = spool.tile([P, NCH, P], f16, tag="xt_sb", name="xt_sb")
                nc.scalar.copy(out=xt_sb, in_=xt_ps)
                q2_ps = ps_q2.tile([P, D], f32, name="q2_ps")
                for j in range(NCH):
                    nc.tensor.matmul(
                        q2_ps, xt_sb[:, j, :], wt_sb[:, j, :],
                        start=(j == 0), stop=False,
                    )
                nc.tensor.matmul(q2_ps, ones_sb, cvec_sb, start=False, stop=True)
                q2_sb = spool.tile([P, D], f16, tag="q2_sb", name="q2_sb")
                nc.scalar.mul(out=q2_sb, in_=q2_ps, mul=INV_SQRT_D)
                q2s[t] = q2_sb

            def emit_merges(t):
                # out rows = 0.5*x + (1/denom)*comb, one stt+store per
                # half-block. Deferred to after tile t+1's score muls so the
                # stt's wait on combine(t) never head-of-line-blocks the DVE
                # queue. Stores go via Pool/SWDGE so their wait on out_sb
                # can't block the SP load queue; the very last store rides
                # the (by then idle) SP queue.
                comb_ps, out_sb, rhalf_sb, xh, b0 = merge_args[t]
                for beta in (0, 1):
                    r0, r1 = 64 * beta, 64 * (beta + 1)
                    nc.vector.scalar_tensor_tensor(
                        out=out_sb[r0:r1, :],
                        in0=comb_ps[r0:r1, :],
                        scalar=rhalf_sb[r0:r1, :],
                        in1=xh[r0:r1, :],
                        op0=mult,
                        op1=add,
                    )
                    eng = nc.sync if (t == NBT - 1 and beta == 1) else nc.gpsimd
                    eng.dma_start(
                        out=out_d[b0 + r0 : b0 + r1, :],
                        in_=out_sb[r0:r1, :],
                    )

            scores_sbs = [None] * NBT

            def scores_section(t):
                # scores[b,k] = q2 . keys[b,k], chasing this tile's key-chunk
                # arrivals (they stream one window ahead of its values). DVE
                # does the fp16 product (2x mode); the length-512 reduce is
                # split 20/32 on ACT (activation accumulator) and 12/32 on
                # DVE (tensor_reduce) to balance engine load.
                q2_sb = q2s[t]
                scores_sb = spool.tile([P, K], f32, tag="scores", name="scores")
                scores_sbs[t] = scores_sb
                for c in range(NKC):
                    ktile = k_tiles[t][c]
                    for kl in range(KC):
                        k = c * KC + kl
                        scratch = scrpool.tile([P, D], f16, tag="scratch",
                                               name="scratch")
                        # Last tile: every 4th product on the (otherwise
                        # idle) Pool engine — its scores are the program's
                        # critical tail and DVE is the pacing engine.
                        eng = (
                            nc.gpsimd
                            if (t == NBT - 1 and k % 4 == 3)
                            else nc.vector
                        )
                        eng.tensor_mul(scratch, q2_sb, ktile[:, kl, :])
                        if k % 8 < 5:
                            nc.scalar.activation(
                                out=scratch, in_=scratch,
                                func=mybir.ActivationFunctionType.Copy,
                                accum_out=scores_sb[:, k : k + 1],
                            )
                        else:
                            nc.vector.tensor_reduce(
                                out=scores_sb[:, k : k + 1],
                                in_=scratch,
                                axis=mybir.AxisListType.X,
                                op=add,
                            )

            load_tile_kx(0)
            q2_section(0)
            scores_section(0)

            for t in range(NBT):
                b0 = t * P
                x_half = x_halves[t]
                scores_sb = scores_sbs[t]

                # next tile's loads + q2 (PE/ACT run it during exp(t)'s wait)
                if t + 1 < NBT:
                    load_tile_kx(t + 1)
                    q2_section(t + 1)

                # --- softmax pieces: E = exp(scores), denom = sum_k E ---
                # Emitted before scores(t+1) so ACT fires exp(t) immediately
                # (tile t's scores finished last window).
                e_sb = spool.tile([P, K], f16, tag="e_sb")
                denom_sb = spool.tile([P, 1], f32, tag="denom")
                nc.scalar.activation(
                    out=e_sb, in_=scores_sb,
                    func=mybir.ActivationFunctionType.Exp,
                    accum_out=denom_sb,
                )
                rhalf_sb = spool.tile([P, 1], f32, tag="rhalf")
                nc.vector.reciprocal(out=rhalf_sb, in_=denom_sb)

                # --- shear E into block-diagonal G2 via PE ---
                # L2[32s+k, 32s+c] = E[4c+s, k] (zeros elsewhere); scattered
                # into G2 col 1024*beta + 68*j + s <- L2 col 32*s + 16*beta
                # + j with ONE strided-AP copy per half (the pattern is
                # affine in (j, s)). The 0.5 output weight folds into the
                # scatter (comb = 0.5*sum E*v, rhalf = 1/denom).
                et_ps = ps_et.tile([K, P], f16)
                et_tr = nc.tensor.transpose(et_ps, e_sb, ident_sb)
                if t == NBT - 1 and comb_insts[t - 1] is not None:
                    # Pin the last tile's shear behind the previous tile's
                    # combine in the in-order PE stream: the list scheduler
                    # otherwise hoists it (its v1 DMA timing runs late) and
                    # the exp(t) wait stalls PE for ~3.5us.
                    tile.add_dep_helper(
                        et_tr.ins, comb_insts[t - 1].ins, sync=True,
                        reason="last-tile shear after prev combine",
                    )
                et_sb = spool.tile([K, P], f16, tag="et_sb")
                nc.scalar.copy(out=et_sb, in_=et_ps)
                et_view = et_sb.rearrange("k (g s4) -> k s4 g", s4=4)
                l2_ps = ps_l2.tile([P, P], f32)
                for s in range(4):
                    nc.tensor.matmul(
                        l2_ps[:, 32 * s : 32 * (s + 1)],
                        smat_sb[:, s, :],
                        et_view[:, s, :],
                        start=True, stop=True,
                    )
                g2_pstr = g2_sb.ap[0][0]
                l2_pstr = l2_ps.ap[0][0]
                for beta in (0, 1):
                    nc.scalar.mul(
                        out=AP(
                            g2_sb.tensor,
                            g2_sb.offset + 1024 * beta,
                            [[g2_pstr, P], [68, NG // 2], [1, 4]],
                        ),
                        in_=AP(
                            l2_ps.tensor,
                            l2_ps.offset + 16 * beta,
                            [[l2_pstr, P], [1, NG // 2], [32, 4]],
                        ),
                        mul=ALPHA,
                    )

                # next tile's scores after this tile's exp/shear (ACT order),
                # then the previous tile's deferred merges+stores (DVE order:
                # after scores(t+1) muls so their wait on combine can't
                # head-of-line-block the queue).
                if t + 1 < NBT:
                    scores_section(t + 1)
                if t > 0:
                    emit_merges(t - 1)

                # --- combine = sum_k E * values via block-diag matmuls ---
                # Half-block beta accumulates its 16 groups into rows
                # [64*beta, 64*beta+64) of comb_ps; each half merges+stores as
                # soon as it is final. The last tile tapers its final chunks.
                comb_ps = ps_cb.tile([P, D], f32)
                out_sb = opool.tile([P, D], f32)
                if t == NBT - 1:
                    vchunks = [(0, 8), (8, 16), (16, 24), (24, 31), (31, 32)]
                else:
                    vchunks = [(0, 8), (8, 16), (16, 24), (24, 32)]
                for vc0, vc1 in vchunks:
                    vtile = vpool.tile([P, vc1 - vc0, D], f16)
                    nc.sync.dma_start(
                        out=vtile,
                        in_=vflat2[:, NG * t + vc0 : NG * t + vc1, :],
                    )
                    for gi in range(vc1 - vc0):
                        g = vc0 + gi
                        beta, j = divmod(g, NG // 2)
                        comb_insts[t] = nc.tensor.matmul(
                            comb_ps[64 * beta : 64 * (beta + 1), :],
                            g2_sb[
                                :, 1024 * beta + 64 * j : 1024 * beta + 64 * (j + 1)
                            ],
                            vtile[:, gi, :],
                            start=(j == 0), stop=(j == NG // 2 - 1),
                        )
                merge_args[t] = (comb_ps, out_sb, rhalf_sb, x_half, b0)
                if t == NBT - 1:
                    emit_merges(t)

    nc.compile()
    return nc


def _get_nc():
    if "nc" not in _NC_CACHE:
        _NC_CACHE["nc"] = _build_nc()
    return _NC_CACHE["nc"]


def _host_consts(w1, b1, w2, b2):
    w1 = np.asarray(w1, np.float32)
    b1 = np.asarray(b1, np.float32)
    w2 = np.asarray(w2, np.float32)
    b2 = np.asarray(b2, np.float32)
    W = w1.T @ w2                       # [D, D]
    c = b1 @ w2                         # [D]
    wt = W.reshape(NCH, P, D).transpose(1, 0, 2).reshape(P, NCH * D)
    identm = np.eye(P, dtype=np.float32)
    wtpack = np.ascontiguousarray(
        np.concatenate([wt, identm], axis=1), np.float16
    )
    cvec = np.ascontiguousarray(c.reshape(1, D), np.float16)
    smat = np.zeros((K, 4, P), np.float16)
    for k in range(K):
        for s in range(4):
            smat[k, s, 32 * s + k] = 1.0
    return wtpack, cvec, smat


def kernel(x, keys, values, w1, b1, w2, b2):
    global LAST_RESULTS
    from concourse import bass_utils

    x = np.ascontiguousarray(np.asarray(x, np.float16))
    keys = np.ascontiguousarray(np.asarray(keys, np.float16))
    values = np.ascontiguousarray(np.asarray(values, np.float16))
    wtpack, cvec, smat = _host_consts(w1, b1, w2, b2)

    nc = _get_nc()
    in_maps = []
    for ci in range(N_CORES):
        sl = slice(ci * BS, (ci + 1) * BS)
        in_maps.append(
            dict(
                xs=x[sl],
                keys=keys[sl],
                values=values[sl],
                wtpack=wtpack,
                cvec=cvec,
                smat=smat,
            )
        )
    res = bass_utils.run_bass_kernel_spmd(
        nc, in_maps, core_ids=list(range(N_CORES))
    )
    LAST_RESULTS = res
    return np.concatenate([r["out"] for r in res.results], axis=0)


# revision 37
# speedup vs baseline: 2.0624x; 1.0010x over previous
"""Trainium2 Bass kernel for retrieval-KNN attention (nn_MAM_68418829025563).

Math (reference):
    query  = x @ w1.T + b1                       # [B, D]
    key    = keys @ w2.T + b2                    # [B, K, D]
    scores = (query . key) / sqrt(D)             # [B, K]
    attn   = softmax(scores, axis=-1)
    out    = 0.5*x + 0.5 * sum_k attn[:,k] * values[:,k,:]

Algebraic refactors:
  - key projection folded into the query side: scores[b,k] ∝ q2[b].keys[b,k]
    with q2 = x @ W + c, W = w1.T @ w2, c = b1 @ w2 (host-folded). The
    s0[b] = x.u + c0 term is constant across k, and softmax is
    shift-invariant, so it is dropped entirely.
  - The kernel is memory-bound (streams keys+values); inputs are converted
    to fp16 on the host, halving HBM traffic. rel-err stays ~1e-3, well
    inside the 2e-2 budget.

Device mapping (pure data parallel, batch sharded over 8 cores):
  - SP queue: all streaming loads (keys prefetched one b-tile ahead, then
    values); Pool/SWDGE: output stores (their long waits must not block
    the load queue). PE: transposes, q2 projection, shear, and the
    block-diagonal attention-weighted value combine (fp16, 1 cycle/row).
  - DVE: fp16 products (2x mode) + 12/32 of the score reductions; ACT:
    20/32 reductions via the activation accumulator, exp, PSUM->SBUF moves.
  - Output merge+store is split per 64-row half-block so the last store
    chain after the final value chunk is short; the last tile's value
    chunks taper (8,8,8,7,1).
"""

import math

import numpy as np

B, K, D = 8192, 32, 512
N_CORES = 8
BS = B // N_CORES        # samples per core
P = 128                  # partition tile (samples per b-tile)
NBT = BS // P            # b-tiles per core
NCH = D // P             # contraction chunks of 128
KC = 8                   # keys per DMA chunk
NKC = K // KC            # key chunks per tile
GC = 8                   # value groups (4 samples each) per DMA chunk
NG = P // 4              # groups of 4 samples per b-tile = 32
INV_SQRT_D = 1.0 / math.sqrt(D)
ALPHA = 0.5

_NC_CACHE = {}
LAST_RESULTS = None


def _build_nc():
    import concourse.bass as bass
    import concourse.tile as tile
    from concourse import bacc, mybir
    from concourse.ap import AP

    f32 = mybir.dt.float32
    f16 = mybir.dt.float16
    nc = bacc.Bacc(
        "TRN2",
        target_bir_lowering=False,
        debug=False,
        enable_asserts=False,
        num_devices=N_CORES,
    )

    xs = nc.dram_tensor("xs", [BS, D], f16, kind="ExternalInput").ap()
    keys = nc.dram_tensor("keys", [BS, K, D], f16, kind="ExternalInput").ap()
    values = nc.dram_tensor("values", [BS, K, D], f16, kind="ExternalInput").ap()
    # wt [P, NCH*D] | ident [P, P] packed in one fp16 tensor (fewer startup DMAs)
    wtpack = nc.dram_tensor(
        "wtpack", [P, NCH * D + P], f16, kind="ExternalInput"
    ).ap()
    cvec = nc.dram_tensor("cvec", [1, D], f16, kind="ExternalInput").ap()
    smat = nc.dram_tensor("smat", [K, 4, P], f16, kind="ExternalInput").ap()
    out_d = nc.dram_tensor("out", [BS, D], f32, kind="ExternalOutput").ap()

    # values as [(b k), d] rows, partition-major view: vflat2[p, n, d] is flat
    # row n*128+p, so tile n holds 4 consecutive samples' (k, d) rows.
    vflat2 = values.flatten_outer_dims().rearrange("(n p) d -> p n d", p=P)

    mult = mybir.AluOpType.mult
    add = mybir.AluOpType.add

    with tile.TileContext(nc) as tc:
        with (
            tc.tile_pool(name="singles", bufs=1) as singles,
            tc.tile_pool(name="kpool", bufs=3 * NKC) as kpool,
            tc.tile_pool(name="vpool", bufs=8) as vpool,
            tc.tile_pool(name="xpool", bufs=3) as xpool,
            tc.tile_pool(name="spool", bufs=2) as spool,
            tc.tile_pool(name="scrpool", bufs=8) as scrpool,
            tc.tile_pool(name="opool", bufs=2) as opool,
            tc.tile_pool(name="ps_xt", bufs=1, space="PSUM") as ps_xt,
            tc.tile_pool(name="ps_q2", bufs=1, space="PSUM") as ps_q2,
            tc.tile_pool(name="ps_et", bufs=1, space="PSUM") as ps_et,
            tc.tile_pool(name="ps_l2", bufs=1, space="PSUM") as ps_l2,
            tc.tile_pool(name="ps_cb", bufs=3, space="PSUM") as ps_cb,
        ):
            # --- constants (loaded once) ---
            wtpack_sb = singles.tile([P, NCH * D + P], f16)
            nc.scalar.dma_start(out=wtpack_sb, in_=wtpack)
            wt_sb = wtpack_sb[:, 0 : NCH * D].rearrange("p (c d) -> p c d", c=NCH)
            ident_sb = wtpack_sb[:, NCH * D : NCH * D + P]
            cvec_sb = singles.tile([1, D], f16)
            nc.scalar.dma_start(out=cvec_sb, in_=cvec)
            smat_sb = singles.tile([K, 4, P], f16)
            nc.sync.dma_start(out=smat_sb, in_=smat)
            ones_sb = singles.tile([1, P], f16)
            nc.vector.memset(ones_sb, 1.0)
            # G2: per-group zero-padded [128, 64] lhsT tiles for the combine.
            # Group g = 16*beta + j lives at cols [1024*beta + 64*j, +64);
            # its only nonzero columns are 4*j + s (s = 0..3), so the single
            # memset here stays valid across all b-tiles.
            g2_sb = singles.tile([P, 2 * NG * P // 4], f16)  # [128, 2048]
            nc.vector.memset(g2_sb, 0.0)

            # Streaming loads, all on the SP queue. Keys (and x) for tile t+1
            # are issued BEFORE tile t's values, so softmax(t) is finished
            # by the time tile t's values arrive and the combine can consume
            # each value chunk immediately.
            x_tiles = [None] * NBT
            k_tiles = [[None] * NKC for _ in range(NBT)]
            x_halves = [None] * NBT
            q2s = [None] * NBT
            merge_args = [None] * NBT
            comb_insts = [None] * NBT

            def load_tile_kx(t):
                b0 = t * P
                x_tiles[t] = xpool.tile([P, D], f16, tag="x", name="x_tile")
                nc.sync.dma_start(out=x_tiles[t], in_=xs[b0 : b0 + P, :])
                for c in range(NKC):
                    kt = kpool.tile([P, KC, D], f16, name="ktile")
                    nc.sync.dma_start(
                        out=kt, in_=keys[b0 : b0 + P, c * KC : (c + 1) * KC, :]
                    )
                    k_tiles[t][c] = kt

            def q2_section(t):
                # xT via PE transpose; q2 = (x @ W + c)/sqrt(D) as fp16.
                # Emitted one tile ahead of tile t-1's shear+combine so the
                # in-order PE queue never gates the next tile's scores.
                x_tile = x_tiles[t]
                x_half = xpool.tile([P, D], f32, tag="xh", name="x_half")
                nc.scalar.mul(out=x_half, in_=x_tile, mul=ALPHA)
                x_halves[t] = x_half
                xt_ps = ps_xt.tile([P, NCH, P], f16, name="xt_ps")
                for j in range(NCH):
                    nc.tensor.transpose(
                        xt_ps[:, j, :], x_tile[:, j * P : (j + 1) * P], ident_sb
                    )
                xt_sb = spool.tile([P, NCH, P], f16, tag="xt_sb", name="xt_sb")
                nc.scalar.copy(out=xt_sb, in_=xt_ps)
                q2_ps = ps_q2.tile([P, D], f32, name="q2_ps")
                for j in range(NCH):
                    nc.tensor.matmul(
                        q2_ps, xt_sb[:, j, :], wt_sb[:, j, :],
                        start=(j == 0), stop=False,
                    )
                nc.tensor.matmul(q2_ps, ones_sb, cvec_sb, start=False, stop=True)
                q2_sb = spool.tile([P, D], f16, tag="q2_sb", name="q2_sb")
                nc.scalar.mul(out=q2_sb, in_=q2_ps, mul=INV_SQRT_D)
                q2s[t] = q2_sb

            def emit_merges(t):
                # out rows = 0.5*x + (1/denom)*comb, one stt+store per
                # half-block. Deferred to after tile t+1's score muls so the
                # stt's wait on combine(t) never head-of-line-blocks the DVE
                # queue. Stores go via Pool/SWDGE so their wait on out_sb
                # can't block the SP load queue; the very last store rides
                # the (by then idle) SP queue.
                comb_ps, out_sb, rhalf_sb, xh, b0 = merge_args[t]
                for beta in (0, 1):
                    r0, r1 = 64 * beta, 64 * (beta + 1)
                    nc.vector.scalar_tensor_tensor(
                        out=out_sb[r0:r1, :],
                        in0=comb_ps[r0:r1, :],
                        scalar=rhalf_sb[r0:r1, :],
                        in1=xh[r0:r1, :],
                        op0=mult,
                        op1=add,
                    )
                    eng = nc.sync if (t == NBT - 1 and beta == 1) else nc.gpsimd
                    eng.dma_start(
                        out=out_d[b0 + r0 : b0 + r1, :],
                        in_=out_sb[r0:r1, :],
                    )

            scores_sbs = [None] * NBT

            def scores_section(t):
                # scores[b,k] = q2 . keys[b,k], chasing this tile's key-chunk
                # arrivals (they stream one window ahead of its values). DVE
                # does the fp16 product (2x mode); the length-512 reduce is
                # split 20/32 on ACT (activation accumulator) and 12/32 on
                # DVE (tensor_reduce) to balance engine load.
                q2_sb = q2s[t]
                scores_sb = spool.tile([P, K], f32, tag="scores", name="scores")
                scores_sbs[t] = scores_sb
                last = t == NBT - 1
                for c in range(NKC):
                    ktile = k_tiles[t][c]
                    for kl in range(KC):
                        k = c * KC + kl
                        scratch = scrpool.tile([P, D], f16, tag="scratch",
                                               name="scratch")
                        # Last tile: its scores are the program's critical
                        # tail — offload 12/32 products to the (otherwise
                        # idle) Pool engine and rebalance the reduces 16/16
                        # so ACT and DVE finish together.
                        eng = (
                            nc.gpsimd
                            if (last and k % 8 in (3, 6, 7))
                            else nc.vector
                        )
                        eng.tensor_mul(scratch, q2_sb, ktile[:, kl, :])
                        if k % 8 < (4 if last else 5):
                            nc.scalar.activation(
                                out=scratch, in_=scratch,
                                func=mybir.ActivationFunctionType.Copy,
                                accum_out=scores_sb[:, k : k + 1],
                            )
                        else:
                            nc.vector.tensor_reduce(
                                out=scores_sb[:, k : k + 1],
                                in_=scratch,
                                axis=mybir.AxisListType.X,
                                op=add,
                            )

            load_tile_kx(0)
            q2_section(0)
            scores_section(0)

            for t in range(NBT):
                b0 = t * P
                x_half = x_halves[t]
                scores_sb = scores_sbs[t]

                # next tile's loads + q2 (PE/ACT run it during exp(t)'s wait)
                if t + 1 < NBT:
                    load_tile_kx(t + 1)
                    q2_section(t + 1)

                # --- softmax pieces: E = exp(scores), denom = sum_k E ---
                # Emitted before scores(t+1) so ACT fires exp(t) immediately
                # (tile t's scores finished last window).
                e_sb = spool.tile([P, K], f16, tag="e_sb")
                denom_sb = spool.tile([P, 1], f32, tag="denom")
                nc.scalar.activation(
                    out=e_sb, in_=scores_sb,
                    func=mybir.ActivationFunctionType.Exp,
                    accum_out=denom_sb,
                )
                rhalf_sb = spool.tile([P, 1], f32, tag="rhalf")
                nc.vector.reciprocal(out=rhalf_sb, in_=denom_sb)

                # --- shear E into block-diagonal G2 via PE ---
                # L2[32s+k, 32s+c] = E[4c+s, k] (zeros elsewhere); scattered
                # into G2 col 1024*beta + 68*j + s <- L2 col 32*s + 16*beta
                # + j with ONE strided-AP copy per half (the pattern is
                # affine in (j, s)). The 0.5 output weight folds into the
                # scatter (comb = 0.5*sum E*v, rhalf = 1/denom).
                et_ps = ps_et.tile([K, P], f16)
                et_tr = nc.tensor.transpose(et_ps, e_sb, ident_sb)
                if t == NBT - 1 and comb_insts[t - 1] is not None:
                    # Pin the last tile's shear behind the previous tile's
                    # combine in the in-order PE stream: the list scheduler
                    # otherwise hoists it (its v1 DMA timing runs late) and
                    # the exp(t) wait stalls PE for ~3.5us.
                    tile.add_dep_helper(
                        et_tr.ins, comb_insts[t - 1].ins, sync=True,
                        reason="last-tile shear after prev combine",
                    )
                et_sb = spool.tile([K, P], f16, tag="et_sb")
                nc.scalar.copy(out=et_sb, in_=et_ps)
                et_view = et_sb.rearrange("k (g s4) -> k s4 g", s4=4)
                l2_ps = ps_l2.tile([P, P], f32)
                for s in range(4):
                    nc.tensor.matmul(
                        l2_ps[:, 32 * s : 32 * (s + 1)],
                        smat_sb[:, s, :],
                        et_view[:, s, :],
                        start=True, stop=True,
                    )
                g2_pstr = g2_sb.ap[0][0]
                l2_pstr = l2_ps.ap[0][0]
                for beta in (0, 1):
                    nc.scalar.mul(
                        out=AP(
                            g2_sb.tensor,
                            g2_sb.offset + 1024 * beta,
                            [[g2_pstr, P], [68, NG // 2], [1, 4]],
                        ),
                        in_=AP(
                            l2_ps.tensor,
                            l2_ps.offset + 16 * beta,
                            [[l2_pstr, P], [1, NG // 2], [32, 4]],
                        ),
                        mul=ALPHA,
                    )

                # next tile's scores after this tile's exp/shear (ACT order),
                # then the previous tile's deferred merges+stores (DVE order:
                # after scores(t+1) muls so their wait on combine can't
                # head-of-line-block the queue).
                if t + 1 < NBT:
                    scores_section(t + 1)
                if t > 0:
                    emit_merges(t - 1)

                # --- combine = sum_k E * values via block-diag matmuls ---
                # Half-block beta accumulates its 16 groups into rows
                # [64*beta, 64*beta+64) of comb_ps; each half merges+stores as
                # soon as it is final. The last tile tapers its final chunks.
                comb_ps = ps_cb.tile([P, D], f32)
                out_sb = opool.tile([P, D], f32)
                if t == NBT - 1:
                    vchunks = [(0, 8), (8, 16), (16, 24), (24, 31), (31, 32)]
                else:
                    vchunks = [(0, 8), (8, 16), (16, 24), (24, 32)]
                for vc0, vc1 in vchunks:
                    vtile = vpool.tile([P, vc1 - vc0, D], f16)
                    nc.sync.dma_start(
                        out=vtile,
                        in_=vflat2[:, NG * t + vc0 : NG * t + vc1, :],
                    )
                    for gi in range(vc1 - vc0):
                        g = vc0 + gi
                        beta, j = divmod(g, NG // 2)
                        comb_insts[t] = nc.tensor.matmul(
                            comb_ps[64 * beta : 64 * (beta + 1), :],
                            g2_sb[
                                :, 1024 * beta + 64 * j : 1024 * beta + 64 * (j + 1)
                            ],
                            vtile[:, gi, :],
                            start=(j == 0), stop=(j == NG // 2 - 1),
                        )
                merge_args[t] = (comb_ps, out_sb, rhalf_sb, x_half, b0)
                if t == NBT - 1:
                    emit_merges(t)

    nc.compile()
    return nc


def _get_nc():
    if "nc" not in _NC_CACHE:
        _NC_CACHE["nc"] = _build_nc()
    return _NC_CACHE["nc"]


def _host_consts(w1, b1, w2, b2):
    w1 = np.asarray(w1, np.float32)
    b1 = np.asarray(b1, np.float32)
    w2 = np.asarray(w2, np.float32)
    b2 = np.asarray(b2, np.float32)
    W = w1.T @ w2                       # [D, D]
    c = b1 @ w2                         # [D]
    wt = W.reshape(NCH, P, D).transpose(1, 0, 2).reshape(P, NCH * D)
    identm = np.eye(P, dtype=np.float32)
    wtpack = np.ascontiguousarray(
        np.concatenate([wt, identm], axis=1), np.float16
    )
    cvec = np.ascontiguousarray(c.reshape(1, D), np.float16)
    smat = np.zeros((K, 4, P), np.float16)
    for k in range(K):
        for s in range(4):
            smat[k, s, 32 * s + k] = 1.0
    return wtpack, cvec, smat


def kernel(x, keys, values, w1, b1, w2, b2):
    global LAST_RESULTS
    from concourse import bass_utils

    x = np.ascontiguousarray(np.asarray(x, np.float16))
    keys = np.ascontiguousarray(np.asarray(keys, np.float16))
    values = np.ascontiguousarray(np.asarray(values, np.float16))
    wtpack, cvec, smat = _host_consts(w1, b1, w2, b2)

    nc = _get_nc()
    in_maps = []
    for ci in range(N_CORES):
        sl = slice(ci * BS, (ci + 1) * BS)
        in_maps.append(
            dict(
                xs=x[sl],
                keys=keys[sl],
                values=values[sl],
                wtpack=wtpack,
                cvec=cvec,
                smat=smat,
            )
        )
    res = bass_utils.run_bass_kernel_spmd(
        nc, in_maps, core_ids=list(range(N_CORES))
    )
    LAST_RESULTS = res
    return np.concatenate([r["out"] for r in res.results], axis=0)
